# revision 1
# baseline (speedup 1.0000x reference)
"""Trainium2 Bass kernel for nn_AttentionKernel_89455578841177.

Multi-head attention: qkv = node @ W_qkv; softmax(q k^T / sqrt(D)) v; out @ W_out.
B=2, S=2048, E=1024, H=16, D=64.

Sharding over 8 NeuronCores: data parallel on B (2) x tensor parallel on heads
(16 heads -> 4 groups of 4). Each core computes a per-head-group partial of the
output projection; the host sums the 4 partials per batch element.

Device-side structure per core (all matmuls bf16 inputs, fp32 PSUM accumulate):
  phase 1: qT = (x Wq)^T, kT = (x Wk)^T in [d, s] layout (weights stationary).
  main loop over q-blocks (hf) x head pairs (mc), ScalarE-exp-bound:
    s^T = k q^T (two K=64 row-tiled matmuls run concurrently) -> exp -> p^T
    [o^T | r] accumulated over k-chunks in PSUM via [v | ones] stationary.
    v itself is projected inside the first q-block's k-loop (x stationary).
    After each (hf): batched approx-reciprocal of the 4 row-sum vectors,
    broadcast across partitions via a DRAM bounce, in-place scale of o^T,
    then that q-block's slice of the output projection y = a W_out.
The 1/sqrt(D) scale is folded into Wq on the host (exact: power of two).
Softmax skips the max-subtraction: scores are ~N(0,1) so exp cannot overflow.
"""

import os

import numpy as np
import ml_dtypes

import concourse.bass as bass
import concourse.mybir as mybir
import concourse.tile as tile
from concourse import bacc
from concourse.bass_utils import run_bass_kernel_spmd

B, S, E = 2, 2048, 1024
H, D = 16, 64
NCORES = 8
GH = 4            # heads per core
GD = GH * D       # 256 = per-core slice of the head dim
P = 128
EO = E // P       # 8 contraction chunks for the projections
SC = S // P       # 16 s-chunks
MC = GD // P      # 2 head-pair chunks (2 heads of 64 rows per chunk)
NQ = 512          # matmul moving free dim
QB = 512          # Sq block size in the attention loop
NHF = S // QB     # 4 q-blocks
KV = D + 1        # v columns + ones column

BF = mybir.dt.bfloat16
FP = mybir.dt.float32
EXP = mybir.ActivationFunctionType.Exp


def _build_kernel(nc: bass.Bass, tc: tile.TileContext):
    xT = nc.dram_tensor("xT", [E, S], BF, kind="ExternalInput")
    wq = nc.dram_tensor("wq", [E, GD], BF, kind="ExternalInput")
    wk = nc.dram_tensor("wk", [E, GD], BF, kind="ExternalInput")
    wv = nc.dram_tensor("wv", [E, GD], BF, kind="ExternalInput")
    wo = nc.dram_tensor("wo", [GD, E], BF, kind="ExternalInput")
    y = nc.dram_tensor("y", [S, E], FP, kind="ExternalOutput")

    with (
        tc.tile_pool(name="const", bufs=1) as const,
        tc.tile_pool(name="pwork", bufs=6) as pwork,
        tc.tile_pool(name="evac", bufs=3) as evac,
    ):
        # ---- SBUF residents -------------------------------------------------
        # weights before x, and x split across both HWDGE rings (sync+scalar),
        # so the first kT matmul isn't queued behind the whole 4 MB x load
        wk_sb = const.tile([P, EO, GD], BF, tag="wk")
        wk_r = wk.rearrange("(eo p) m -> p eo m", p=P)
        nc.sync.dma_start(out=wk_sb[:, : EO // 2], in_=wk_r[:, : EO // 2])
        nc.sync.dma_start(out=wk_sb[:, EO // 2 :], in_=wk_r[:, EO // 2 :])
        wq_sb = const.tile([P, EO, GD], BF, tag="wq")
        wq_r = wq.rearrange("(eo p) m -> p eo m", p=P)
        nc.scalar.dma_start(out=wq_sb[:, : EO // 2], in_=wq_r[:, : EO // 2])
        nc.scalar.dma_start(out=wq_sb[:, EO // 2 :], in_=wq_r[:, EO // 2 :])
        wv_sb = const.tile([P, EO, GD], BF, tag="wv")
        nc.scalar.dma_start(out=wv_sb, in_=wv.rearrange("(eo p) m -> p eo m", p=P))

        x_sb = const.tile([P, EO, S], BF, tag="x")
        xT_r = xT.rearrange("(eo p) s -> p eo s", p=P)
        for sh in range(2):  # s-halves: the first kT/qT sweep needs only half
            for eo in range(EO):
                eng = nc.sync if eo % 2 == 0 else nc.scalar
                eng.dma_start(
                    out=x_sb[:, eo, sh * (S // 2) : (sh + 1) * (S // 2)],
                    in_=xT_r[:, eo, sh * (S // 2) : (sh + 1) * (S // 2)],
                )

        wo_sb = const.tile([P, MC, E], BF, tag="wo")
        nc.sync.dma_start(out=wo_sb, in_=wo.rearrange("(mc p) e -> p mc e", p=P))

        qT_sb = const.tile([P, MC, S], BF, tag="qT")
        kT_sb = const.tile([P, MC, S], BF, tag="kT")
        at_sb = const.tile([P, MC, S], BF, tag="at")   # attn out^T (unnorm->norm)
        v_sb = const.tile([P, SC, GH, KV], BF, tag="v")
        nc.vector.memset(v_sb[:, :, :, D : D + 1], 1.0)
        # ones column for broadcasting 1/r rows across partitions via K=1 mm
        ones_b = const.tile([1, 64], BF, tag="ones")
        nc.vector.memset(ones_b, 1.0)

        # one PSUM bank budget for everything: scores pair (2 banks x2 bufs),
        # [o^T|r] accumulators (1 bank x2), and a shared 1-bank pool for the
        # projections / broadcasts (x2) = 8 banks exactly
        with (
            tc.tile_pool(name="ps_sc", bufs=2, space="PSUM") as ps_sc,
            tc.tile_pool(name="ps_pv", bufs=2, space="PSUM") as ps_pv,
            tc.tile_pool(name="psq", bufs=2, space="PSUM") as psq,
        ):
            def emit_proj(wsrc, dst, mc, sqb):
                """One [s-block 1024] x [128 dims] projection group."""
                psts = [
                    psq.tile([P, NQ], FP, tag="sq", name=f"pst{sq}")
                    for sq in range(2)
                ]
                for eo in range(EO):
                    for sq in range(2):
                        s0 = (sqb * 2 + sq) * NQ
                        nc.tensor.matmul(
                            psts[sq],
                            lhsT=wsrc[:, eo, mc * P : (mc + 1) * P],
                            rhs=x_sb[:, eo, s0 : s0 + NQ],
                            start=(eo == 0),
                            stop=(eo == EO - 1),
                        )
                for sq in range(2):
                    s0 = (sqb * 2 + sq) * NQ
                    nc.vector.tensor_copy(out=dst[:, mc, s0 : s0 + NQ], in_=psts[sq])

            def emit_attention(hf, mc, r4, with_v=False):
                q0 = hf * QB
                po = [
                    ps_pv.tile([KV, QB], FP, tag="po", name=f"po{h}")
                    for h in range(2)
                ]
                for kc in range(SC):
                    # head pair packed side by side, one fp32 bank per head;
                    # the K=64 row-tiled matmuls run concurrently
                    st = ps_sc.tile([P, 2 * QB], FP, tag="st")
                    for h in range(2):
                        hb = h * 64
                        nc.tensor.matmul(
                            st[:, h * QB : (h + 1) * QB],
                            lhsT=kT_sb[hb : hb + 64, mc, kc * P : (kc + 1) * P],
                            rhs=qT_sb[hb : hb + 64, mc, q0 : q0 + QB],
                            start=True,
                            stop=True,
                        )
                    pt = pwork.tile([P, 2 * QB], BF, tag="p")
                    nc.scalar.activation(pt, st, EXP)
                    if with_v:
                        # v projection for this k-chunk (all 4 heads)
                        psv = psq.tile([P, NQ], FP, tag="sq", name="psv")
                        for eo in range(EO):
                            nc.tensor.matmul(
                                psv[:, :GD],
                                lhsT=x_sb[:, eo, kc * P : (kc + 1) * P],
                                rhs=wv_sb[:, eo, :],
                                start=(eo == 0),
                                stop=(eo == EO - 1),
                            )
                        nc.vector.tensor_copy(
                            out=v_sb[:, kc, :, 0:D],
                            in_=psv[:, :GD].rearrange("p (h d) -> p h d", h=GH),
                        )
                    for h in range(2):
                        nc.tensor.matmul(
                            po[h],
                            lhsT=v_sb[:, kc, mc * 2 + h, :],
                            rhs=pt[:, h * QB : (h + 1) * QB],
                            start=(kc == 0),
                            stop=(kc == SC - 1),
                            skip_group_check=True,
                        )
                # evacuate PSUM immediately so the po slots recycle
                for h in range(2):
                    hb = h * 64
                    nc.vector.tensor_copy(
                        out=at_sb[hb : hb + 64, mc, q0 : q0 + QB],
                        in_=po[h][0:D, :],
                    )
                    nc.vector.tensor_copy(
                        out=r4[0:1, mc * 2 + h, :], in_=po[h][D : D + 1, :]
                    )

            def emit_norm_outproj(hf, r4):
                # normalization, all on-chip: batched approx reciprocal on the
                # partition-0 staging rows, cast to bf16, then broadcast each
                # row across 64 partitions with a K=1 ones matmul
                q0 = hf * QB
                rinv4 = evac.tile([1, 4, QB], FP, tag="rinv4", bufs=2)
                nc.vector.reciprocal_approx_fast(rinv4, r4)
                rinvb = evac.tile([1, 4, QB], BF, tag="rinvb", bufs=2)
                nc.vector.tensor_copy(out=rinvb, in_=rinv4)
                for mc in range(MC):
                    rb_ps = psq.tile([P, QB], FP, tag="sq", name="rb")
                    for h in range(2):
                        nc.tensor.matmul(
                            rb_ps[h * 64 : (h + 1) * 64, :],
                            lhsT=ones_b,
                            rhs=rinvb[0:1, mc * 2 + h, :],
                            start=True,
                            stop=True,
                        )
                    nc.vector.tensor_tensor(
                        at_sb[:, mc, q0 : q0 + QB],
                        at_sb[:, mc, q0 : q0 + QB],
                        rb_ps,
                        mybir.AluOpType.mult,
                    )
                # output projection for this q-block
                for sc in range(hf * (QB // P), (hf + 1) * (QB // P)):
                    y_sb = evac.tile([P, E], FP, tag="ysb")
                    for nq in range(E // NQ):
                        psy = psq.tile([P, NQ], FP, tag="sq", name="psy")
                        for mc in range(MC):
                            nc.tensor.matmul(
                                psy,
                                lhsT=at_sb[:, mc, sc * P : (sc + 1) * P],
                                rhs=wo_sb[:, mc, nq * NQ : (nq + 1) * NQ],
                                start=(mc == 0),
                                stop=(mc == MC - 1),
                            )
                        nc.vector.tensor_copy(
                            out=y_sb[:, nq * NQ : (nq + 1) * NQ], in_=psy
                        )
                    nc.sync.dma_start(out=y[sc * P : (sc + 1) * P, :], in_=y_sb)

            # emission order = per-engine schedule order: start attention as
            # early as possible and backfill the remaining projections into
            # the PE's exp-wait slack
            r4s = {
                hf: evac.tile([1, 4, QB], FP, tag="r4", bufs=2, name=f"r4_{hf}")
                for hf in range(NHF)
            }
            emit_proj(wk_sb, kT_sb, 0, 0)
            emit_proj(wq_sb, qT_sb, 0, 0)
            emit_proj(wk_sb, kT_sb, 0, 1)
            emit_attention(0, 0, r4s[0], with_v=True)
            emit_proj(wk_sb, kT_sb, 1, 0)
            emit_proj(wq_sb, qT_sb, 1, 0)
            emit_proj(wk_sb, kT_sb, 1, 1)
            emit_attention(0, 1, r4s[0])
            emit_norm_outproj(0, r4s[0])
            emit_proj(wq_sb, qT_sb, 0, 1)
            emit_proj(wq_sb, qT_sb, 1, 1)
            for hf in range(1, NHF):
                emit_attention(hf, 0, r4s[hf])
                emit_attention(hf, 1, r4s[hf])
                emit_norm_outproj(hf, r4s[hf])


_NC_CACHE = None


def build_nc() -> bass.Bass:
    global _NC_CACHE
    if _NC_CACHE is None:
        nc = bacc.Bacc(None, target_bir_lowering=False)
        with tile.TileContext(nc) as tc:
            _build_kernel(nc, tc)
        nc.compile()
        _NC_CACHE = nc
    return _NC_CACHE


def make_core_inputs(node: np.ndarray, W_qkv: np.ndarray, W_out: np.ndarray):
    """Shard full inputs into the 8 per-core input maps."""
    bf16 = ml_dtypes.bfloat16
    in_maps = []
    for c in range(NCORES):
        b, g = divmod(c, NCORES // B)
        sl = slice(g * GD, (g + 1) * GD)
        in_maps.append(
            {
                "xT": np.ascontiguousarray(node[b].T).astype(bf16),
                # fold the 1/sqrt(D) softmax scale into Wq (exact in bf16)
                "wq": np.ascontiguousarray(W_qkv[:, sl] * (1.0 / np.sqrt(D))).astype(
                    bf16
                ),
                "wk": np.ascontiguousarray(
                    W_qkv[:, H * D + g * GD : H * D + (g + 1) * GD]
                ).astype(bf16),
                "wv": np.ascontiguousarray(
                    W_qkv[:, 2 * H * D + g * GD : 2 * H * D + (g + 1) * GD]
                ).astype(bf16),
                "wo": np.ascontiguousarray(W_out[sl, :]).astype(bf16),
            }
        )
    return in_maps


def _run(node, W_qkv, W_out, **spmd_kwargs):
    nc = build_nc()
    in_maps = make_core_inputs(node, W_qkv, W_out)
    res = run_bass_kernel_spmd(
        nc, in_maps, core_ids=list(range(NCORES)), **spmd_kwargs
    )
    out = np.zeros((B, S, E), dtype=np.float32)
    for c in range(NCORES):
        b = c // (NCORES // B)
        out[b] += res.results[c]["y"]
    return out, res


def kernel(node: np.ndarray, W_qkv: np.ndarray, W_out: np.ndarray) -> np.ndarray:
    node = np.asarray(node, dtype=np.float32)
    W_qkv = np.asarray(W_qkv, dtype=np.float32)
    W_out = np.asarray(W_out, dtype=np.float32)
    out, _ = _run(node, W_qkv, W_out)
    return out



# revision 2
# speedup vs baseline: 1.0375x; 1.0375x over previous
"""Trainium2 Bass kernel for nn_AttentionKernel_89455578841177.

Multi-head attention: qkv = node @ W_qkv; softmax(q k^T / sqrt(D)) v; out @ W_out.
B=2, S=2048, E=1024, H=16, D=64.

Sharding over 8 NeuronCores: data parallel on B (2) x tensor parallel on heads
(16 heads -> 4 groups of 4). Each core computes a per-head-group partial of the
output projection; the host sums the 4 partials per batch element.

v2 schedule: the Act engine's exp stream (128 x [128,1024] activations,
~145us) is the hard floor; everything else is scheduled around keeping it
saturated from ~10us on.
  - x streams in 512-column s-blocks (order sb1,sb0,sb2,sb3) across both HWDGE
    rings so the first scores block (hf=1, whose queries live in sb1) can start
    after ~2MB of DMA instead of the full 6MB.
  - Attention runs as two sweeps: mc=0 over hf [1,2,3,0], then mc=1 over hf
    [0,1,2,3].  All projections (kT/qT both mc), the v projection, and the
    norm + output projection of earlier blocks are emitted as small backfill
    thunks at explicit iteration slots inside the sweeps, sized to the PE
    slack of each window.
  - The first window necessarily carries the whole v projection JIT (its
    att*v needs every v chunk), so the Act stream lags ~10us there; every
    later window is Act-bound.
The 1/sqrt(D) scale is folded into Wq on the host (exact: power of two).
Softmax skips the max-subtraction: scores are ~N(0,1) so exp cannot overflow.
"""

import numpy as np
import ml_dtypes

import concourse.bass as bass
import concourse.mybir as mybir
import concourse.tile as tile
from concourse import bacc
from concourse.bass_utils import run_bass_kernel_spmd

B, S, E = 2, 2048, 1024
H, D = 16, 64
NCORES = 8
GH = 4            # heads per core
GD = GH * D       # 256 = per-core slice of the head dim
P = 128
EO = E // P       # 8 contraction chunks for the projections
SC = S // P       # 16 s-chunks (key chunks)
MC = GD // P      # 2 head-pair chunks (2 heads of 64 rows per chunk)
NQ = 512          # matmul moving free dim / s-block size
QB = 512          # Sq block size in the attention loop
NHF = S // QB     # 4 q-blocks
KV = D + 1        # v columns + ones column

BF = mybir.dt.bfloat16
FP = mybir.dt.float32
EXP = mybir.ActivationFunctionType.Exp

SBORD = (1, 0, 2, 3)                                  # x s-block DMA order
KCORD = (4, 5, 6, 7, 0, 1, 2, 3, 8, 9, 10, 11, 12, 13, 14, 15)


def _build_kernel(nc: bass.Bass, tc: tile.TileContext):
    xT = nc.dram_tensor("xT", [E, S], BF, kind="ExternalInput")
    wq = nc.dram_tensor("wq", [E, GD], BF, kind="ExternalInput")
    wk = nc.dram_tensor("wk", [E, GD], BF, kind="ExternalInput")
    wv = nc.dram_tensor("wv", [E, GD], BF, kind="ExternalInput")
    wo = nc.dram_tensor("wo", [GD, E], BF, kind="ExternalInput")
    y = nc.dram_tensor("y", [S, E], FP, kind="ExternalOutput")

    with (
        tc.tile_pool(name="const", bufs=1) as const,
        tc.tile_pool(name="pwork", bufs=6) as pwork,
        tc.tile_pool(name="evac", bufs=3) as evac,
    ):
        # ---- SBUF residents -------------------------------------------------
        wk_sb = const.tile([P, EO, GD], BF, tag="wk")
        wq_sb = const.tile([P, EO, GD], BF, tag="wq")
        wv_sb = const.tile([P, EO, GD], BF, tag="wv")
        wo_sb = const.tile([P, MC, E], BF, tag="wo")
        x_sb = const.tile([P, EO, S], BF, tag="x")
        qT_sb = const.tile([P, MC, S], BF, tag="qT")
        kT_sb = const.tile([P, MC, S], BF, tag="kT")
        at_sb = const.tile([P, MC, S], BF, tag="at")   # attn out^T (unnorm->norm)
        v_sb = const.tile([P, SC, GH, KV], BF, tag="v")
        r4_all = const.tile([1, NHF, GH, QB], FP, tag="r4all")  # softmax row sums
        ones_b = const.tile([1, 64], BF, tag="ones")

        # ---- DMA emission: ring A = sync, ring B = scalar -------------------
        # Weight prefixes first (kT/qT gate on them), then x s-blocks in SBORD
        # order split even/odd-eo across the rings; wv after sb0 (v projection
        # starts mid-first-window); wo last (first out-proj is ~100us in).
        wk_r = wk.rearrange("(eo p) m -> p eo m", p=P)
        wq_r = wq.rearrange("(eo p) m -> p eo m", p=P)
        wv_r = wv.rearrange("(eo p) m -> p eo m", p=P)
        nc.sync.dma_start(out=wk_sb, in_=wk_r)
        nc.scalar.dma_start(out=wq_sb, in_=wq_r)
        xT_r = xT.rearrange("(eo p) s -> p eo s", p=P)
        for sb in SBORD:
            s0 = sb * NQ
            for eo in range(EO):
                eng = nc.sync if eo % 2 == 0 else nc.scalar
                eng.dma_start(
                    out=x_sb[:, eo, s0 : s0 + NQ], in_=xT_r[:, eo, s0 : s0 + NQ]
                )
            if sb == 0:
                nc.scalar.dma_start(out=wv_sb, in_=wv_r)
        nc.sync.dma_start(out=wo_sb, in_=wo.rearrange("(mc p) e -> p mc e", p=P))
        nc.vector.memset(v_sb[:, :, :, D : D + 1], 1.0)
        nc.vector.memset(ones_b, 1.0)

        # scores pair (2 banks x2 bufs) + [o^T|r] accumulators (1 bank x2) +
        # shared 1-bank pool for projections / broadcasts (x2) = 8 banks
        with (
            tc.tile_pool(name="ps_sc", bufs=2, space="PSUM") as ps_sc,
            tc.tile_pool(name="ps_pv", bufs=2, space="PSUM") as ps_pv,
            tc.tile_pool(name="psq", bufs=2, space="PSUM") as psq,
        ):
            def proj_block(wsrc, dst, mc, sb):
                """One 512-col s-block of a q/k projection (8 matmuls)."""
                s0 = sb * NQ
                pst = psq.tile([P, NQ], FP, tag="sq", name=f"pj{mc}{sb}")
                for eo in range(EO):
                    nc.tensor.matmul(
                        pst,
                        lhsT=wsrc[:, eo, mc * P : (mc + 1) * P],
                        rhs=x_sb[:, eo, s0 : s0 + NQ],
                        start=(eo == 0),
                        stop=(eo == EO - 1),
                    )
                nc.vector.tensor_copy(out=dst[:, mc, s0 : s0 + NQ], in_=pst)

            def v_block(kc):
                """v projection for one 128-key chunk (all 4 heads)."""
                psv = psq.tile([P, NQ], FP, tag="sq", name=f"v{kc}")
                for eo in range(EO):
                    nc.tensor.matmul(
                        psv[:, :GD],
                        lhsT=x_sb[:, eo, kc * P : (kc + 1) * P],
                        rhs=wv_sb[:, eo, :],
                        start=(eo == 0),
                        stop=(eo == EO - 1),
                    )
                nc.vector.tensor_copy(
                    out=v_sb[:, kc, :, 0:D],
                    in_=psv[:, :GD].rearrange("p (h d) -> p h d", h=GH),
                )

            def window(hf, mc, due, kcord=KCORD):
                """One attention block: 16x (scores pair -> exp -> att*v pair),
                with backfill thunks emitted at the given iteration slots."""
                q0 = hf * QB
                po = [
                    ps_pv.tile([KV, QB], FP, tag="po", name=f"po{hf}{mc}{h}")
                    for h in range(2)
                ]
                n = len(kcord)
                for i, kc in enumerate(kcord):
                    st = ps_sc.tile([P, 2 * QB], FP, tag="st")
                    for h in range(2):
                        hb = h * 64
                        nc.tensor.matmul(
                            st[:, h * QB : (h + 1) * QB],
                            lhsT=kT_sb[hb : hb + 64, mc, kc * P : (kc + 1) * P],
                            rhs=qT_sb[hb : hb + 64, mc, q0 : q0 + QB],
                            start=True,
                            stop=True,
                        )
                    pt = pwork.tile([P, 2 * QB], BF, tag="p")
                    nc.scalar.activation(pt, st, EXP)
                    for th in due.get(i, ()):
                        th()
                    for h in range(2):
                        nc.tensor.matmul(
                            po[h],
                            lhsT=v_sb[:, kc, mc * 2 + h, :],
                            rhs=pt[:, h * QB : (h + 1) * QB],
                            start=(i == 0),
                            stop=(i == n - 1),
                            skip_group_check=True,
                        )
                for h in range(2):
                    hb = h * 64
                    nc.vector.tensor_copy(
                        out=at_sb[hb : hb + 64, mc, q0 : q0 + QB], in_=po[h][0:D, :]
                    )
                    nc.vector.tensor_copy(
                        out=r4_all[0:1, hf, mc * 2 + h, :], in_=po[h][D : D + 1, :]
                    )

            def norm(hf):
                """Batched reciprocal of the 4 row-sum vectors, broadcast across
                partitions via K=1 ones matmuls, in-place scale of at^T."""
                q0 = hf * QB
                rinv4 = evac.tile([1, GH, QB], FP, tag="rinv4", bufs=2)
                nc.vector.reciprocal_approx_fast(rinv4, r4_all[0:1, hf])
                rinvb = evac.tile([1, GH, QB], BF, tag="rinvb", bufs=2)
                nc.vector.tensor_copy(out=rinvb, in_=rinv4)
                for mc in range(MC):
                    rb_ps = psq.tile([P, QB], FP, tag="sq", name=f"rb{hf}{mc}")
                    for h in range(2):
                        nc.tensor.matmul(
                            rb_ps[h * 64 : (h + 1) * 64, :],
                            lhsT=ones_b,
                            rhs=rinvb[0:1, mc * 2 + h, :],
                            start=True,
                            stop=True,
                        )
                    nc.vector.tensor_tensor(
                        at_sb[:, mc, q0 : q0 + QB],
                        at_sb[:, mc, q0 : q0 + QB],
                        rb_ps,
                        mybir.AluOpType.mult,
                    )

            def outproj(hf, sc_i):
                """Output projection for one 128-row slice of a q-block."""
                sc = hf * (QB // P) + sc_i
                y_sb = evac.tile([P, E], FP, tag="ysb")
                for nq in range(E // NQ):
                    psy = psq.tile([P, NQ], FP, tag="sq", name=f"py{sc}{nq}")
                    for mc in range(MC):
                        nc.tensor.matmul(
                            psy,
                            lhsT=at_sb[:, mc, sc * P : (sc + 1) * P],
                            rhs=wo_sb[:, mc, nq * NQ : (nq + 1) * NQ],
                            start=(mc == 0),
                            stop=(mc == MC - 1),
                        )
                    nc.vector.tensor_copy(out=y_sb[:, nq * NQ : (nq + 1) * NQ], in_=psy)
                nc.sync.dma_start(out=y[sc * P : (sc + 1) * P, :], in_=y_sb)

            def PB(w, d, m, s):
                return lambda: proj_block(w, d, m, s)

            def VB(kc):
                return lambda: v_block(kc)

            def NM(hf):
                return lambda: norm(hf)

            def OP(hf, sc_i):
                return lambda: outproj(hf, sc_i)

            # ---- pre-phase: just enough for the first scores block ----------
            proj_block(wk_sb, kT_sb, 0, 1)
            proj_block(wq_sb, qT_sb, 0, 1)

            # ---- sweep 1: mc=0 over hf [1,2,3,0] ----------------------------
            # window(1,0) carries the whole v projection JIT (each VB(kc)
            # before that kc's att*v) plus the kT(0,*) blocks its own scores
            # need; later windows carry the remaining projections.
            window(1, 0, {
                0: [VB(4)], 1: [VB(5)], 2: [VB(6)],
                3: [VB(7), PB(wk_sb, kT_sb, 0, 0)],
                4: [VB(0)], 5: [VB(1)], 6: [VB(2)],
                7: [VB(3), PB(wk_sb, kT_sb, 0, 2)],
                8: [VB(8)], 9: [VB(9)], 10: [VB(10)],
                11: [VB(11), PB(wk_sb, kT_sb, 0, 3)],
                12: [VB(12)], 13: [VB(13), PB(wq_sb, qT_sb, 0, 2)],
                14: [VB(14)], 15: [VB(15)],
            })
            window(2, 0, {
                1: [PB(wq_sb, qT_sb, 0, 3)],
                3: [PB(wk_sb, kT_sb, 1, 1)],
                5: [PB(wk_sb, kT_sb, 1, 0)],
                7: [PB(wq_sb, qT_sb, 1, 0)],
            })
            window(3, 0, {
                1: [PB(wq_sb, qT_sb, 0, 0)],
                3: [PB(wk_sb, kT_sb, 1, 2)],
                5: [PB(wk_sb, kT_sb, 1, 3)],
            })
            window(0, 0, {
                1: [PB(wq_sb, qT_sb, 1, 1)],
                3: [PB(wq_sb, qT_sb, 1, 2)],
                5: [PB(wq_sb, qT_sb, 1, 3)],
            })

            # ---- sweep 2: mc=1 over hf [0,1,2,3], norm/outproj backfilled ---
            nat = tuple(range(SC))
            window(0, 1, {}, kcord=nat)
            window(1, 1, {
                1: [NM(0)], 3: [OP(0, 0)], 5: [OP(0, 1)],
                7: [OP(0, 2)], 9: [OP(0, 3)],
            }, kcord=nat)
            window(2, 1, {
                1: [NM(1)], 3: [OP(1, 0)], 5: [OP(1, 1)],
                7: [OP(1, 2)], 9: [OP(1, 3)],
            }, kcord=nat)
            window(3, 1, {
                1: [NM(2)], 3: [OP(2, 0)], 5: [OP(2, 1)],
                7: [OP(2, 2)], 9: [OP(2, 3)],
            }, kcord=nat)

            # ---- tail -------------------------------------------------------
            norm(3)
            for sc_i in range(QB // P):
                outproj(3, sc_i)


_NC_CACHE = None


def build_nc() -> bass.Bass:
    global _NC_CACHE
    if _NC_CACHE is None:
        nc = bacc.Bacc(None, target_bir_lowering=False)
        with tile.TileContext(nc) as tc:
            _build_kernel(nc, tc)
        nc.compile()
        _NC_CACHE = nc
    return _NC_CACHE


def make_core_inputs(node: np.ndarray, W_qkv: np.ndarray, W_out: np.ndarray):
    """Shard full inputs into the 8 per-core input maps."""
    bf16 = ml_dtypes.bfloat16
    in_maps = []
    for c in range(NCORES):
        b, g = divmod(c, NCORES // B)
        sl = slice(g * GD, (g + 1) * GD)
        in_maps.append(
            {
                "xT": np.ascontiguousarray(node[b].T).astype(bf16),
                # fold the 1/sqrt(D) softmax scale into Wq (exact in bf16)
                "wq": np.ascontiguousarray(W_qkv[:, sl] * (1.0 / np.sqrt(D))).astype(
                    bf16
                ),
                "wk": np.ascontiguousarray(
                    W_qkv[:, H * D + g * GD : H * D + (g + 1) * GD]
                ).astype(bf16),
                "wv": np.ascontiguousarray(
                    W_qkv[:, 2 * H * D + g * GD : 2 * H * D + (g + 1) * GD]
                ).astype(bf16),
                "wo": np.ascontiguousarray(W_out[sl, :]).astype(bf16),
            }
        )
    return in_maps


def _run(node, W_qkv, W_out, **spmd_kwargs):
    nc = build_nc()
    in_maps = make_core_inputs(node, W_qkv, W_out)
    res = run_bass_kernel_spmd(
        nc, in_maps, core_ids=list(range(NCORES)), **spmd_kwargs
    )
    out = np.zeros((B, S, E), dtype=np.float32)
    for c in range(NCORES):
        b = c // (NCORES // B)
        out[b] += res.results[c]["y"]
    return out, res


def kernel(node: np.ndarray, W_qkv: np.ndarray, W_out: np.ndarray) -> np.ndarray:
    node = np.asarray(node, dtype=np.float32)
    W_qkv = np.asarray(W_qkv, dtype=np.float32)
    W_out = np.asarray(W_out, dtype=np.float32)
    out, _ = _run(node, W_qkv, W_out)
    return out


# revision 9
# speedup vs baseline: 1.0969x; 1.0572x over previous
"""Trainium2 Bass kernel for nn_AttentionKernel_89455578841177.

Multi-head attention: qkv = node @ W_qkv; softmax(q k^T / sqrt(D)) v; out @ W_out.
B=2, S=2048, E=1024, H=16, D=64.

Sharding over 8 NeuronCores: data parallel on B (2) x tensor parallel on heads
(16 heads -> 4 groups of 4). Each core computes a per-head-group partial of the
output projection; the host sums the 4 partials per batch element.

v3 schedule: the Act engine's exp stream (128 x [128,1024] activations,
~145us busy) is the hard floor; everything is arranged to keep it saturated.
  - All DRAM operands are pre-arranged on the host so each load is ONE
    contiguous DMA dispatch (dispatch instructions cost ~0.6-0.9us each on the
    issuing engine, so dispatch count on the critical prefix matters more than
    transfer shape).  x is staged in 512-column s-blocks, parity-split across
    the two HWDGE rings (sync + scalar), in order sb1,sb0,sb2,sb3.
  - One flat software pipeline over all 128 (hf,mc,kc) iterations: scores ->
    exp -> backfill micro-thunks -> att*v of the PREVIOUS iteration.  The
    lag-1 att*v keeps window boundaries off the exp critical path.
  - All projections (kT/qT both mc), the v projection, and norm + output
    projection of earlier blocks are emitted as <=4-matmul micro-thunks at
    explicit iteration slots, sized to the PE slack of each window.
  - y is stored as bf16 (host upcasts and reduces across head-groups).
The 1/sqrt(D) scale is folded into Wq on the host (exact: power of two).
Softmax skips the max-subtraction: scores are ~N(0,1) so exp cannot overflow.
"""

import numpy as np
import ml_dtypes

import concourse.bass as bass
import concourse.mybir as mybir
import concourse.tile as tile
from concourse import bacc
from concourse.bass_utils import run_bass_kernel_spmd

B, S, E = 2, 2048, 1024
H, D = 16, 64
NCORES = 8
GH = 4            # heads per core
GD = GH * D       # 256 = per-core slice of the head dim
P = 128
EO = E // P       # 8 contraction chunks for the projections
SC = S // P       # 16 s-chunks (key chunks)
MC = GD // P      # 2 head-pair chunks (2 heads of 64 rows per chunk)
NQ = 512          # matmul moving free dim / s-block size
QB = 512          # Sq block size in the attention loop
NHF = S // QB     # 4 q-blocks
KV = D + 1        # v columns + ones column

BF = mybir.dt.bfloat16
FP = mybir.dt.float32
EXP = mybir.ActivationFunctionType.Exp

SBORD = (1, 0, 2, 3)                                  # x s-block DMA order
KCORD = (4, 5, 6, 7, 0, 1, 2, 3, 8, 9, 10, 11, 12, 13, 14, 15)


def _build_kernel(nc: bass.Bass, tc: tile.TileContext):
    # Host pre-arranged layouts (see make_core_inputs):
    #   xa: [SBORD-index][parity][128][4*512]  (parity = eo%2, 4 eo per ring)
    #   wk/wq/wv: [128][EO*GD],  wo: [128][MC*E]
    xa = nc.dram_tensor("xa", [NHF, 2, P, 4 * NQ], BF, kind="ExternalInput")
    wq = nc.dram_tensor("wq", [P, EO * GD], BF, kind="ExternalInput")
    wk = nc.dram_tensor("wk", [P, EO * GD], BF, kind="ExternalInput")
    wv = nc.dram_tensor("wv", [P, EO * GD], BF, kind="ExternalInput")
    wo = nc.dram_tensor("wo", [P, MC * E], BF, kind="ExternalInput")
    y = nc.dram_tensor("y", [S, E], BF, kind="ExternalOutput")

    with (
        tc.tile_pool(name="const", bufs=1) as const,
        tc.tile_pool(name="pwork", bufs=6) as pwork,
        tc.tile_pool(name="evac", bufs=3) as evac,
    ):
        # ---- SBUF residents -------------------------------------------------
        wk_sb = const.tile([P, EO, GD], BF, tag="wk")
        wq_sb = const.tile([P, EO, GD], BF, tag="wq")
        wv_sb = const.tile([P, EO, GD], BF, tag="wv")
        wo_sb = const.tile([P, MC, E], BF, tag="wo")
        x_sb = const.tile([P, EO, S], BF, tag="x")
        qT_sb = const.tile([P, MC, S], BF, tag="qT")
        kT_sb = const.tile([P, MC, S], BF, tag="kT")
        at_sb = const.tile([P, MC, S], BF, tag="at")   # attn out^T (unnorm->norm)
        v_sb = const.tile([P, SC, GH, KV], BF, tag="v")
        r4_all = const.tile([1, NHF, GH, QB], FP, tag="r4all")  # softmax row sums
        ones_b = const.tile([1, 64], BF, tag="ones")

        # ---- DMA emission: ring A = sync, ring B = scalar -------------------
        # One dispatch per operand / (s-block, parity).  Ring B (the Act
        # engine's queue) gets as few dispatches as possible.
        nc.sync.dma_start(out=wk_sb.rearrange("p eo m -> p (eo m)"), in_=wk[:, :])
        nc.scalar.dma_start(out=wq_sb.rearrange("p eo m -> p (eo m)"), in_=wq[:, :])
        x_par = x_sb.rearrange("p (eo2 par) s -> p par eo2 s", par=2)
        for si, sb in enumerate(SBORD):
            s0 = sb * NQ
            for par, eng in ((0, nc.sync), (1, nc.scalar)):
                eng.dma_start(
                    out=x_par[:, par, :, s0 : s0 + NQ],
                    in_=xa[si, par].rearrange("p (eo s) -> p eo s", eo=4),
                )
            if sb == 1:
                nc.sync.dma_start(
                    out=wv_sb.rearrange("p eo m -> p (eo m)"), in_=wv[:, :]
                )
        nc.sync.dma_start(out=wo_sb.rearrange("p mc e -> p (mc e)"), in_=wo[:, :])
        nc.vector.memset(v_sb[:, :, :, D : D + 1], 1.0)
        nc.vector.memset(ones_b, 1.0)

        # scores pair (2 banks x2 bufs) + [o^T|r] accumulators (1 bank x2) +
        # shared 1-bank pool for projections / broadcasts (x2) = 8 banks
        with (
            tc.tile_pool(name="ps_sc", bufs=2, space="PSUM") as ps_sc,
            tc.tile_pool(name="ps_pv", bufs=2, space="PSUM") as ps_pv,
            tc.tile_pool(name="psq", bufs=2, space="PSUM") as psq,
        ):
            # ---- micro-thunk generators (each thunk <= ~4 matmuls) ---------
            def proj_thunks(wsrc, dst, mc, sb):
                """q/k projection s-block as 2 thunks sharing one PSUM group."""
                s0 = sb * NQ
                box = {}

                def half(lo):
                    if lo == 0:
                        box["t"] = psq.tile([P, NQ], FP, tag="sq", name=f"pj{mc}{sb}")
                    pst = box["t"]
                    for eo in range(lo, lo + 4):
                        nc.tensor.matmul(
                            pst,
                            lhsT=wsrc[:, eo, mc * P : (mc + 1) * P],
                            rhs=x_sb[:, eo, s0 : s0 + NQ],
                            start=(eo == 0),
                            stop=(eo == EO - 1),
                        )
                    if lo == 4:
                        nc.vector.tensor_copy(out=dst[:, mc, s0 : s0 + NQ], in_=pst)

                return [lambda: half(0), lambda: half(4)]

            def v_thunks(kc):
                """v projection for one 128-key chunk as 2 thunks."""
                box = {}

                def half(lo):
                    if lo == 0:
                        box["t"] = psq.tile([P, NQ], FP, tag="sq", name=f"v{kc}")
                    psv = box["t"]
                    for eo in range(lo, lo + 4):
                        nc.tensor.matmul(
                            psv[:, :GD],
                            lhsT=x_sb[:, eo, kc * P : (kc + 1) * P],
                            rhs=wv_sb[:, eo, :],
                            start=(eo == 0),
                            stop=(eo == EO - 1),
                        )
                    if lo == 4:
                        nc.vector.tensor_copy(
                            out=v_sb[:, kc, :, 0:D],
                            in_=psv[:, :GD].rearrange("p (h d) -> p h d", h=GH),
                        )

                return [lambda: half(0), lambda: half(4)]

            def norm_thunk(hf, mc):
                """Reciprocal (mc==0 only) + partition-broadcast + scale of
                one head-pair's slice of at^T."""
                q0 = hf * QB

                def run():
                    if mc == 0:
                        rinv4 = evac.tile([1, GH, QB], FP, tag="rinv4", bufs=2)
                        nc.vector.reciprocal_approx_fast(rinv4, r4_all[0:1, hf])
                        rb = evac.tile([1, GH, QB], BF, tag="rinvb", bufs=2)
                        nc.vector.tensor_copy(out=rb, in_=rinv4)
                        norm_thunk.rb = rb
                    rb = norm_thunk.rb
                    rb_ps = psq.tile([P, QB], FP, tag="sq", name=f"rb{hf}{mc}")
                    for h in range(2):
                        nc.tensor.matmul(
                            rb_ps[h * 64 : (h + 1) * 64, :],
                            lhsT=ones_b,
                            rhs=rb[0:1, mc * 2 + h, :],
                            start=True,
                            stop=True,
                        )
                    nc.vector.tensor_tensor(
                        at_sb[:, mc, q0 : q0 + QB],
                        at_sb[:, mc, q0 : q0 + QB],
                        rb_ps,
                        mybir.AluOpType.mult,
                    )

                return run

            def outproj_thunks(hf, sc_i):
                """Output projection for one 128-row q-slice as 2 thunks."""
                sc = hf * (QB // P) + sc_i
                box = {}

                def part(nq):
                    if nq == 0:
                        box["y"] = evac.tile([P, E], BF, tag="ysb", name=f"y{sc}")
                    y_sb = box["y"]
                    psy = psq.tile([P, NQ], FP, tag="sq", name=f"py{sc}{nq}")
                    for mc in range(MC):
                        nc.tensor.matmul(
                            psy,
                            lhsT=at_sb[:, mc, sc * P : (sc + 1) * P],
                            rhs=wo_sb[:, mc, nq * NQ : (nq + 1) * NQ],
                            start=(mc == 0),
                            stop=(mc == MC - 1),
                        )
                    nc.vector.tensor_copy(out=y_sb[:, nq * NQ : (nq + 1) * NQ], in_=psy)
                    if nq == 1:
                        nc.sync.dma_start(out=y[sc * P : (sc + 1) * P, :], in_=y_sb)

                return [lambda: part(0), lambda: part(1)]

            # ---- flat pipelined emission over all windows -------------------
            def scores_exp(hf, mc, kc):
                q0 = hf * QB
                st = ps_sc.tile([P, 2 * QB], FP, tag="st")
                for h in range(2):
                    hb = h * 64
                    nc.tensor.matmul(
                        st[:, h * QB : (h + 1) * QB],
                        lhsT=kT_sb[hb : hb + 64, mc, kc * P : (kc + 1) * P],
                        rhs=qT_sb[hb : hb + 64, mc, q0 : q0 + QB],
                        start=True,
                        stop=True,
                    )
                pt = pwork.tile([P, 2 * QB], BF, tag="p")
                nc.scalar.activation(pt, st, EXP)
                return pt

            def attv(mc, kc, pt, po, first, last):
                for h in range(2):
                    nc.tensor.matmul(
                        po[h],
                        lhsT=v_sb[:, kc, mc * 2 + h, :],
                        rhs=pt[:, h * QB : (h + 1) * QB],
                        start=first,
                        stop=last,
                        skip_group_check=True,
                    )

            def po_evac(hf, mc, po):
                for h in range(2):
                    hb = h * 64
                    nc.vector.tensor_copy(
                        out=at_sb[hb : hb + 64, mc, hf * QB : (hf + 1) * QB],
                        in_=po[h][0:D, :],
                    )
                    nc.vector.tensor_copy(
                        out=r4_all[0:1, hf, mc * 2 + h, :], in_=po[h][D : D + 1, :]
                    )

            def run_windows(windows):
                pend = None  # (hf, mc, kc, pt, po, first, last)
                for hf, mc, kcord, due in windows:
                    po = [
                        ps_pv.tile([KV, QB], FP, tag="po", name=f"po{hf}{mc}{h}")
                        for h in range(2)
                    ]
                    n = len(kcord)
                    for i, kc in enumerate(kcord):
                        pt = scores_exp(hf, mc, kc)
                        for th in due.get(i, ()):
                            th()
                        if pend is not None:
                            attv(pend[1], pend[2], pend[3], pend[4], pend[5], pend[6])
                            if pend[6]:
                                po_evac(pend[0], pend[1], pend[4])
                        pend = (hf, mc, kc, pt, po, i == 0, i == n - 1)
                attv(pend[1], pend[2], pend[3], pend[4], pend[5], pend[6])
                po_evac(pend[0], pend[1], pend[4])

            PJ = proj_thunks
            VB = v_thunks
            nat = tuple(range(SC))

            # ---- pre-phase: just enough for the first scores block ----------
            for th in PJ(wk_sb, kT_sb, 0, 1) + PJ(wq_sb, qT_sb, 0, 1):
                th()

            def merge(*slot_lists):
                out = {}
                for slots in slot_lists:
                    for k, v in slots.items():
                        out.setdefault(k, []).extend(v if isinstance(v, list) else [v])
                return out

            w10 = merge(
                {i: VB(KCORD[i]) for i in range(16)},
                dict(zip((2, 3), PJ(wk_sb, kT_sb, 0, 0))),
                dict(zip((5, 6), PJ(wk_sb, kT_sb, 0, 2))),
                dict(zip((9, 10), PJ(wk_sb, kT_sb, 0, 3))),
                dict(zip((12, 13), PJ(wq_sb, qT_sb, 0, 2))),
            )
            w20 = merge(
                dict(zip((0, 1), PJ(wq_sb, qT_sb, 0, 3))),
                dict(zip((3, 4), PJ(wk_sb, kT_sb, 1, 1))),
                dict(zip((6, 7), PJ(wk_sb, kT_sb, 1, 0))),
                dict(zip((9, 10), PJ(wq_sb, qT_sb, 1, 0))),
            )
            w30 = merge(
                dict(zip((0, 1), PJ(wq_sb, qT_sb, 0, 0))),
                dict(zip((3, 4), PJ(wk_sb, kT_sb, 1, 2))),
                dict(zip((6, 7), PJ(wk_sb, kT_sb, 1, 3))),
            )
            w00 = merge(
                dict(zip((1, 2), PJ(wq_sb, qT_sb, 1, 1))),
                dict(zip((4, 5), PJ(wq_sb, qT_sb, 1, 2))),
                dict(zip((7, 8), PJ(wq_sb, qT_sb, 1, 3))),
            )

            def np_slots(hf):  # norm + outproj of block hf, as 10 slotted thunks
                return merge(
                    {1: [norm_thunk(hf, 0)], 2: [norm_thunk(hf, 1)]},
                    dict(zip((4, 5), outproj_thunks(hf, 0))),
                    dict(zip((7, 8), outproj_thunks(hf, 1))),
                    dict(zip((10, 11), outproj_thunks(hf, 2))),
                    dict(zip((13, 14), outproj_thunks(hf, 3))),
                )

            run_windows([
                # sweep 1: mc=0 over hf [1,2,3,0]; v + projections backfilled
                (1, 0, KCORD, w10),
                (2, 0, KCORD, w20),
                (3, 0, KCORD, w30),
                (0, 0, KCORD, w00),
                # sweep 2: mc=1 over hf [0,1,2,3]; norm/outproj backfilled
                (0, 1, nat, {}),
                (1, 1, nat, np_slots(0)),
                (2, 1, nat, np_slots(1)),
                (3, 1, nat, np_slots(2)),
            ])

            # ---- tail -------------------------------------------------------
            norm_thunk(3, 0)()
            norm_thunk(3, 1)()
            for sc_i in range(QB // P):
                for th in outproj_thunks(3, sc_i):
                    th()


_NC_CACHE = None


def build_nc() -> bass.Bass:
    global _NC_CACHE
    if _NC_CACHE is None:
        nc = bacc.Bacc(None, target_bir_lowering=False)
        with tile.TileContext(nc) as tc:
            _build_kernel(nc, tc)
        nc.compile()
        _NC_CACHE = nc
    return _NC_CACHE


def make_core_inputs(node: np.ndarray, W_qkv: np.ndarray, W_out: np.ndarray):
    """Shard full inputs into the 8 per-core input maps (pre-arranged)."""
    bf16 = ml_dtypes.bfloat16

    def arr_w(w):  # [E, M] -> [128, EO*M], eo-major per partition
        m = w.shape[1]
        return np.ascontiguousarray(
            w.reshape(EO, P, m).transpose(1, 0, 2).reshape(P, EO * m)
        ).astype(bf16)

    in_maps = []
    for c in range(NCORES):
        b, g = divmod(c, NCORES // B)
        sl = slice(g * GD, (g + 1) * GD)
        xT = node[b].T  # [E, S]
        # xa[si][par][p][4*NQ]: s-block SBORD[si], eo = par, par+2, par+4, par+6
        xr = xT.reshape(EO, P, NHF, NQ)
        xa = np.empty((NHF, 2, P, 4 * NQ), dtype=np.float32)
        for si, sb in enumerate(SBORD):
            for par in range(2):
                xa[si, par] = (
                    xr[par::2, :, sb, :].transpose(1, 0, 2).reshape(P, 4 * NQ)
                )
        wox = W_out[sl, :]  # [GD, E]
        in_maps.append(
            {
                "xa": np.ascontiguousarray(xa).astype(bf16),
                # fold the 1/sqrt(D) softmax scale into Wq (exact in bf16)
                "wq": arr_w(W_qkv[:, sl] * (1.0 / np.sqrt(D))),
                "wk": arr_w(W_qkv[:, H * D + g * GD : H * D + (g + 1) * GD]),
                "wv": arr_w(W_qkv[:, 2 * H * D + g * GD : 2 * H * D + (g + 1) * GD]),
                "wo": np.ascontiguousarray(
                    wox.reshape(MC, P, E).transpose(1, 0, 2).reshape(P, MC * E)
                ).astype(bf16),
            }
        )
    return in_maps


def _run(node, W_qkv, W_out, **spmd_kwargs):
    nc = build_nc()
    in_maps = make_core_inputs(node, W_qkv, W_out)
    res = run_bass_kernel_spmd(
        nc, in_maps, core_ids=list(range(NCORES)), **spmd_kwargs
    )
    out = np.zeros((B, S, E), dtype=np.float32)
    for c in range(NCORES):
        b = c // (NCORES // B)
        out[b] += res.results[c]["y"].astype(np.float32)
    return out, res


def kernel(node: np.ndarray, W_qkv: np.ndarray, W_out: np.ndarray) -> np.ndarray:
    node = np.asarray(node, dtype=np.float32)
    W_qkv = np.asarray(W_qkv, dtype=np.float32)
    W_out = np.asarray(W_out, dtype=np.float32)
    out, _ = _run(node, W_qkv, W_out)
    return out


# revision 17
# speedup vs baseline: 1.1381x; 1.0376x over previous
"""Trainium2 Bass kernel for nn_AttentionKernel_89455578841177.

Multi-head attention: qkv = node @ W_qkv; softmax(q k^T / sqrt(D)) v; out @ W_out.
B=2, S=2048, E=1024, H=16, D=64.

Sharding over 8 NeuronCores: data parallel on B (2) x tensor parallel on heads
(16 heads -> 4 groups of 4). Each core computes a per-head-group partial of the
output projection; the host sums the 4 partials per batch element.

v3 schedule: the Act engine's exp stream (128 x [128,1024] activations,
~145us busy) is the hard floor; everything is arranged to keep it saturated.
  - All DRAM operands are pre-arranged on the host so each load is ONE
    contiguous DMA dispatch (dispatch instructions cost ~0.6-0.9us each on the
    issuing engine, so dispatch count on the critical prefix matters more than
    transfer shape).  x is staged in 512-column s-blocks, parity-split across
    the two HWDGE rings (sync + scalar), in order sb1,sb0,sb2,sb3.
  - One flat software pipeline over all 128 (hf,mc,kc) iterations: scores ->
    exp -> backfill micro-thunks -> att*v of the PREVIOUS iteration.  The
    lag-1 att*v keeps window boundaries off the exp critical path.
  - All projections (kT/qT both mc), the v projection, and norm + output
    projection of earlier blocks are emitted as <=4-matmul micro-thunks at
    explicit iteration slots, sized to the PE slack of each window.
  - y is stored as bf16 (host upcasts and reduces across head-groups).
The 1/sqrt(D) scale is folded into Wq on the host (exact: power of two).
Softmax skips the max-subtraction: scores are ~N(0,1) so exp cannot overflow.
"""

import numpy as np
import ml_dtypes

import concourse.bass as bass
import concourse.mybir as mybir
import concourse.tile as tile
from concourse import bacc
from concourse.bass_utils import run_bass_kernel_spmd

B, S, E = 2, 2048, 1024
H, D = 16, 64
NCORES = 8
GH = 4            # heads per core
GD = GH * D       # 256 = per-core slice of the head dim
P = 128
EO = E // P       # 8 contraction chunks for the projections
SC = S // P       # 16 s-chunks (key chunks)
MC = GD // P      # 2 head-pair chunks (2 heads of 64 rows per chunk)
NQ = 512          # matmul moving free dim / s-block size
QB = 512          # Sq block size in the attention loop
NHF = S // QB     # 4 q-blocks
KV = D + 1        # v columns + ones column

BF = mybir.dt.bfloat16
FP = mybir.dt.float32
EXP = mybir.ActivationFunctionType.Exp

SBORD = (1, 0, 2, 3)                                  # x s-block DMA order
KCORD = (4, 5, 6, 7, 0, 1, 2, 3, 8, 9, 10, 11, 12, 13, 14, 15)


def _build_kernel(nc: bass.Bass, tc: tile.TileContext):
    # Host pre-arranged layouts (see make_core_inputs):
    #   xa: [SBORD-index][parity][128][4*512]  (parity = eo%2, 4 eo per ring)
    #   wk/wq/wv: [128][EO*GD],  wo: [128][MC*E]
    xa = nc.dram_tensor("xa", [NHF, 2, P, 4 * NQ], BF, kind="ExternalInput")
    wq = nc.dram_tensor("wq", [P, MC * EO * P], BF, kind="ExternalInput")
    wk = nc.dram_tensor("wk", [P, MC * EO * P], BF, kind="ExternalInput")
    wv = nc.dram_tensor("wv", [P, EO * GD], BF, kind="ExternalInput")
    wo = nc.dram_tensor("wo", [P, MC * E], BF, kind="ExternalInput")
    y = nc.dram_tensor("y", [S, E], BF, kind="ExternalOutput")

    with (
        tc.tile_pool(name="const", bufs=1) as const,
        tc.tile_pool(name="pwork", bufs=6) as pwork,
        tc.tile_pool(name="evac", bufs=3) as evac,
    ):
        # ---- SBUF residents -------------------------------------------------
        wk_sb = const.tile([P, MC, EO, P], BF, tag="wk")
        wq_sb = const.tile([P, MC, EO, P], BF, tag="wq")
        wv_sb = const.tile([P, EO, GD], BF, tag="wv")
        wo_sb = const.tile([P, MC, E], BF, tag="wo")
        x_sb = const.tile([P, EO, S], BF, tag="x")
        qT_sb = const.tile([P, MC, S], BF, tag="qT")
        kT_sb = const.tile([P, MC, S], BF, tag="kT")
        at_sb = const.tile([P, MC, S], BF, tag="at")   # attn out^T (unnorm->norm)
        v_sb = const.tile([P, SC, GH, KV], BF, tag="v")
        # softmax row sums: head-slot g lives on partition 32*g (32-alignment
        # keeps the K=1 broadcast matmuls' tile_position legal)
        r4_sb = const.tile([P, NHF, QB], FP, tag="r4sb")
        ones4 = const.tile([P, 64], BF, tag="ones4")
        junk = const.tile([P, NQ], BF, tag="junk")

        # ---- DMA emission: ring A = sync, ring B = scalar -------------------
        # One dispatch per operand / (s-block, parity).  Ring B (the Act
        # engine's queue) gets as few dispatches as possible.
        HW = EO * P  # flat size of one mc-half of wk/wq
        wk_f = wk_sb.rearrange("p mc eo m -> p (mc eo m)")
        wq_f = wq_sb.rearrange("p mc eo m -> p (mc eo m)")
        nc.sync.dma_start(out=wk_f[:, :HW], in_=wk[:, :HW])
        nc.scalar.dma_start(out=wq_f[:, :HW], in_=wq[:, :HW])
        x_par = x_sb.rearrange("p (eo2 par) s -> p par eo2 s", par=2)
        for si, sb in enumerate(SBORD):
            s0 = sb * NQ
            for par, eng in ((0, nc.sync), (1, nc.scalar)):
                eng.dma_start(
                    out=x_par[:, par, :, s0 : s0 + NQ],
                    in_=xa[si, par].rearrange("p (eo s) -> p eo s", eo=4),
                )
            if sb == 1:
                nc.sync.dma_start(
                    out=wv_sb.rearrange("p eo m -> p (eo m)"), in_=wv[:, :]
                )
            if sb == 2:
                nc.sync.dma_start(out=wk_f[:, HW:], in_=wk[:, HW:])
            if sb == 0:
                nc.scalar.dma_start(out=wq_f[:, HW:], in_=wq[:, HW:])
        nc.sync.dma_start(out=wo_sb.rearrange("p mc e -> p (mc e)"), in_=wo[:, :])
        nc.vector.memset(v_sb[:, :, :, D : D + 1], 1.0)
        nc.vector.memset(ones4, 1.0)
        nc.vector.memset(junk, 0.0)

        # scores pair (2 banks x2 bufs) + [o^T|r] accumulators (1 bank x2) +
        # shared 1-bank pool for projections / broadcasts (x2) = 8 banks
        with (
            tc.tile_pool(name="ps_sc", bufs=2, space="PSUM") as ps_sc,
            tc.tile_pool(name="ps_pv", bufs=2, space="PSUM") as ps_pv,
            tc.tile_pool(name="psq", bufs=2, space="PSUM") as psq,
        ):
            # ---- micro-thunk generators (each thunk <= ~4 matmuls) ---------
            def proj_thunks(wsrc, dst, mc, sb):
                """q/k projection s-block as 2 thunks sharing one PSUM group."""
                s0 = sb * NQ
                box = {}

                def half(lo):
                    if lo == 0:
                        box["t"] = psq.tile([P, NQ], FP, tag="sq", name=f"pj{mc}{sb}")
                    pst = box["t"]
                    for eo in range(lo, lo + 4):
                        nc.tensor.matmul(
                            pst,
                            lhsT=wsrc[:, mc, eo, :],
                            rhs=x_sb[:, eo, s0 : s0 + NQ],
                            start=(eo == 0),
                            stop=(eo == EO - 1),
                        )
                    if lo == 4:
                        nc.vector.tensor_copy(out=dst[:, mc, s0 : s0 + NQ], in_=pst)

                return [lambda: half(0), lambda: half(4)]

            def v_thunks(kc):
                """v projection for one 128-key chunk as 2 thunks."""
                box = {}

                def half(lo):
                    if lo == 0:
                        box["t"] = psq.tile([P, NQ], FP, tag="sq", name=f"v{kc}")
                    psv = box["t"]
                    for eo in range(lo, lo + 4):
                        nc.tensor.matmul(
                            psv[:, :GD],
                            lhsT=x_sb[:, eo, kc * P : (kc + 1) * P],
                            rhs=wv_sb[:, eo, :],
                            start=(eo == 0),
                            stop=(eo == EO - 1),
                        )
                    if lo == 4:
                        nc.vector.tensor_copy(
                            out=v_sb[:, kc, :, 0:D],
                            in_=psv[:, :GD].rearrange("p (h d) -> p h d", h=GH),
                        )

                return [lambda: half(0), lambda: half(4)]

            def norm_thunk(hf, mc):
                """Reciprocal (mc==0 only) + partition-broadcast + scale of
                one head-pair's slice of at^T."""
                q0 = hf * QB

                def run():
                    if mc == 0:
                        # full 128-partition reciprocal: only rows 0/32/64/96
                        # hold real sums, the rest is never read
                        rinv4 = evac.tile([P, QB], FP, tag="rinv4", bufs=2)
                        nc.vector.reciprocal_approx_fast(rinv4, r4_sb[:, hf])
                        rb = evac.tile([P, QB], BF, tag="rinvb", bufs=2)
                        nc.vector.tensor_copy(out=rb, in_=rinv4)
                        norm_thunk.rb = rb
                    rb = norm_thunk.rb
                    rb_ps = psq.tile([P, QB], FP, tag="sq", name=f"rb{hf}{mc}")
                    for h in range(2):
                        g = 32 * (mc * 2 + h)
                        nc.tensor.matmul(
                            rb_ps[h * 64 : (h + 1) * 64, :],
                            lhsT=ones4[g : g + 1, :],
                            rhs=rb[g : g + 1, :],
                            start=True,
                            stop=True,
                            tile_position=(g, h * 64),
                        )
                    nc.vector.tensor_tensor(
                        at_sb[:, mc, q0 : q0 + QB],
                        at_sb[:, mc, q0 : q0 + QB],
                        rb_ps,
                        mybir.AluOpType.mult,
                    )

                return run

            def outproj_thunks(hf, sc_i):
                """Output projection for one 128-row q-slice as 2 thunks."""
                sc = hf * (QB // P) + sc_i
                box = {}

                def part(nq):
                    if nq == 0:
                        box["y"] = evac.tile([P, E], BF, tag="ysb", name=f"y{sc}")
                    y_sb = box["y"]
                    psy = psq.tile([P, NQ], FP, tag="sq", name=f"py{sc}{nq}")
                    for mc in range(MC):
                        nc.tensor.matmul(
                            psy,
                            lhsT=at_sb[:, mc, sc * P : (sc + 1) * P],
                            rhs=wo_sb[:, mc, nq * NQ : (nq + 1) * NQ],
                            start=(mc == 0),
                            stop=(mc == MC - 1),
                        )
                    nc.vector.tensor_copy(out=y_sb[:, nq * NQ : (nq + 1) * NQ], in_=psy)
                    if nq == 1:
                        nc.sync.dma_start(out=y[sc * P : (sc + 1) * P, :], in_=y_sb)

                return [lambda: part(0), lambda: part(1)]

            # ---- flat pipelined emission over all windows -------------------
            def scores_exp(hf, mc, kc):
                q0 = hf * QB
                st = ps_sc.tile([P, 2 * QB], FP, tag="st")
                for h in range(2):
                    hb = h * 64
                    nc.tensor.matmul(
                        st[:, h * QB : (h + 1) * QB],
                        lhsT=kT_sb[hb : hb + 64, mc, kc * P : (kc + 1) * P],
                        rhs=qT_sb[hb : hb + 64, mc, q0 : q0 + QB],
                        start=True,
                        stop=True,
                    )
                pt = pwork.tile([P, 2 * QB], BF, tag="p")
                nc.scalar.activation(pt, st, EXP)
                return pt

            def attv(mc, kc, pt, po, first, last):
                for h in range(2):
                    nc.tensor.matmul(
                        po[h],
                        lhsT=v_sb[:, kc, mc * 2 + h, :],
                        rhs=pt[:, h * QB : (h + 1) * QB],
                        start=first,
                        stop=last,
                        skip_group_check=True,
                    )

            def po_evac(hf, mc, po):
                for h in range(2):
                    hb = h * 64
                    nc.vector.tensor_copy(
                        out=at_sb[hb : hb + 64, mc, hf * QB : (hf + 1) * QB],
                        in_=po[h][0:D, :],
                    )
                    nc.vector.tensor_copy(
                        out=r4_sb[32 * (mc * 2 + h) : 32 * (mc * 2 + h) + 1, hf, :],
                        in_=po[h][D : D + 1, :],
                    )

            def run_windows(windows):
                pend = None  # (hf, mc, kc, pt, po, first, last)
                for hf, mc, kcord, due in windows:
                    po = [
                        ps_pv.tile([KV, QB], FP, tag="po", name=f"po{hf}{mc}{h}")
                        for h in range(2)
                    ]
                    n = len(kcord)
                    for i, kc in enumerate(kcord):
                        pt = scores_exp(hf, mc, kc)
                        for th in due.get(i, ()):
                            th()
                        if pend is not None:
                            attv(pend[1], pend[2], pend[3], pend[4], pend[5], pend[6])
                            if pend[6]:
                                po_evac(pend[0], pend[1], pend[4])
                        pend = (hf, mc, kc, pt, po, i == 0, i == n - 1)
                attv(pend[1], pend[2], pend[3], pend[4], pend[5], pend[6])
                po_evac(pend[0], pend[1], pend[4])

            PJ = proj_thunks
            VB = v_thunks
            nat = tuple(range(SC))

            # ---- PE warm-up: keep the HAM clock ramping while the x DMA
            # streams in (results are never read)
            psj = psq.tile([P, NQ], FP, tag="sq", name="warm")
            for _ in range(24):
                nc.tensor.matmul(
                    psj, lhsT=junk[:, :P], rhs=junk, start=True, stop=True
                )

            # ---- pre-phase: just enough for the first scores block ----------
            for th in PJ(wk_sb, kT_sb, 0, 1) + PJ(wq_sb, qT_sb, 0, 1):
                th()

            def merge(*slot_lists):
                out = {}
                for slots in slot_lists:
                    for k, v in slots.items():
                        out.setdefault(k, []).extend(v if isinstance(v, list) else [v])
                return out

            w1 = merge(   # hf=1: whole v projection JIT + its own kT blocks
                {i: VB(KCORD[i]) for i in range(16)},
                dict(zip((2, 3), PJ(wk_sb, kT_sb, 0, 0))),
                dict(zip((5, 6), PJ(wk_sb, kT_sb, 0, 2))),
                dict(zip((9, 10), PJ(wk_sb, kT_sb, 0, 3))),
                dict(zip((12, 13), PJ(wq_sb, qT_sb, 0, 0))),
            )
            w2 = merge(   # hf=0
                dict(zip((0, 1), PJ(wq_sb, qT_sb, 0, 2))),
                dict(zip((3, 4), PJ(wk_sb, kT_sb, 1, 1))),
                dict(zip((6, 7), PJ(wk_sb, kT_sb, 1, 0))),
                dict(zip((9, 10), PJ(wq_sb, qT_sb, 1, 0))),
            )
            w3 = merge(   # hf=2
                dict(zip((0, 1), PJ(wq_sb, qT_sb, 0, 3))),
                dict(zip((3, 4), PJ(wk_sb, kT_sb, 1, 2))),
                dict(zip((6, 7), PJ(wk_sb, kT_sb, 1, 3))),
            )
            w4 = merge(   # hf=3
                dict(zip((1, 2), PJ(wq_sb, qT_sb, 1, 1))),
                dict(zip((4, 5), PJ(wq_sb, qT_sb, 1, 2))),
                dict(zip((7, 8), PJ(wq_sb, qT_sb, 1, 3))),
            )

            def np_slots(hf):  # norm + outproj of block hf, as slotted thunks
                return merge(
                    {2: [norm_thunk(hf, 0)], 3: [norm_thunk(hf, 1)]},
                    dict(zip((5, 6), outproj_thunks(hf, 0))),
                    dict(zip((8, 9), outproj_thunks(hf, 1))),
                    dict(zip((11, 12), outproj_thunks(hf, 2))),
                    dict(zip((13, 14), outproj_thunks(hf, 3))),
                )

            run_windows([
                # sweep 1: mc=0 over hf [1,0,2,3]; v + projections backfilled
                (1, 0, KCORD, w1),
                (0, 0, KCORD, w2),
                (2, 0, KCORD, w3),
                (3, 0, KCORD, w4),
                # sweep 2: mc=1 over hf [0,1,2,3]; norm/outproj backfilled
                (0, 1, nat, {}),
                (1, 1, nat, np_slots(0)),
                (2, 1, nat, np_slots(1)),
                (3, 1, nat, np_slots(2)),
            ])

            # ---- tail: last block's norm + outproj; evacuations alternate
            # DVE / Act-copy (the exp stream is over, so Act is free)
            norm_thunk(3, 0)()
            norm_thunk(3, 1)()
            COPYF = mybir.ActivationFunctionType.Copy
            for sc_i in range(QB // P):
                sc = 3 * (QB // P) + sc_i
                y_sb = evac.tile([P, E], BF, tag="ysb", name=f"yt{sc}")
                for nq in range(E // NQ):
                    psy = psq.tile([P, NQ], FP, tag="sq", name=f"pyt{sc}{nq}")
                    for mc in range(MC):
                        nc.tensor.matmul(
                            psy,
                            lhsT=at_sb[:, mc, sc * P : (sc + 1) * P],
                            rhs=wo_sb[:, mc, nq * NQ : (nq + 1) * NQ],
                            start=(mc == 0),
                            stop=(mc == MC - 1),
                        )
                    if nq == 0:
                        nc.vector.tensor_copy(out=y_sb[:, :NQ], in_=psy)
                    else:
                        nc.scalar.activation(y_sb[:, NQ:], psy, COPYF)
                nc.sync.dma_start(out=y[sc * P : (sc + 1) * P, :], in_=y_sb)


_NC_CACHE = None


def build_nc() -> bass.Bass:
    global _NC_CACHE
    if _NC_CACHE is None:
        nc = bacc.Bacc(None, target_bir_lowering=False)
        with tile.TileContext(nc) as tc:
            _build_kernel(nc, tc)
        nc.compile()
        _NC_CACHE = nc
    return _NC_CACHE


def make_core_inputs(node: np.ndarray, W_qkv: np.ndarray, W_out: np.ndarray):
    """Shard full inputs into the 8 per-core input maps (pre-arranged)."""
    bf16 = ml_dtypes.bfloat16

    def arr_w(w):  # [E, M] -> [128, EO*M], eo-major per partition
        m = w.shape[1]
        return np.ascontiguousarray(
            w.reshape(EO, P, m).transpose(1, 0, 2).reshape(P, EO * m)
        ).astype(bf16)

    def arr_w_mc(w):  # [E, GD] -> [128, MC*EO*128], mc-major per partition
        return np.ascontiguousarray(
            w.reshape(EO, P, MC, P).transpose(1, 2, 0, 3).reshape(P, MC * EO * P)
        ).astype(bf16)

    in_maps = []
    for c in range(NCORES):
        b, g = divmod(c, NCORES // B)
        sl = slice(g * GD, (g + 1) * GD)
        xT = node[b].T  # [E, S]
        # xa[si][par][p][4*NQ]: s-block SBORD[si], eo = par, par+2, par+4, par+6
        xr = xT.reshape(EO, P, NHF, NQ)
        xa = np.empty((NHF, 2, P, 4 * NQ), dtype=np.float32)
        for si, sb in enumerate(SBORD):
            for par in range(2):
                xa[si, par] = (
                    xr[par::2, :, sb, :].transpose(1, 0, 2).reshape(P, 4 * NQ)
                )
        wox = W_out[sl, :]  # [GD, E]
        in_maps.append(
            {
                "xa": np.ascontiguousarray(xa).astype(bf16),
                # fold the 1/sqrt(D) softmax scale into Wq (exact in bf16)
                "wq": arr_w_mc(W_qkv[:, sl] * (1.0 / np.sqrt(D))),
                "wk": arr_w_mc(W_qkv[:, H * D + g * GD : H * D + (g + 1) * GD]),
                "wv": arr_w(W_qkv[:, 2 * H * D + g * GD : 2 * H * D + (g + 1) * GD]),
                "wo": np.ascontiguousarray(
                    wox.reshape(MC, P, E).transpose(1, 0, 2).reshape(P, MC * E)
                ).astype(bf16),
            }
        )
    return in_maps


def _run(node, W_qkv, W_out, **spmd_kwargs):
    nc = build_nc()
    in_maps = make_core_inputs(node, W_qkv, W_out)
    res = run_bass_kernel_spmd(
        nc, in_maps, core_ids=list(range(NCORES)), **spmd_kwargs
    )
    out = np.zeros((B, S, E), dtype=np.float32)
    for c in range(NCORES):
        b = c // (NCORES // B)
        out[b] += res.results[c]["y"].astype(np.float32)
    return out, res


def kernel(node: np.ndarray, W_qkv: np.ndarray, W_out: np.ndarray) -> np.ndarray:
    node = np.asarray(node, dtype=np.float32)
    W_qkv = np.asarray(W_qkv, dtype=np.float32)
    W_out = np.asarray(W_out, dtype=np.float32)
    out, _ = _run(node, W_qkv, W_out)
    return out
